# revision 30
# baseline (speedup 1.0000x reference)
"""Discriminator-loss kernel for Trainium2, SPMD across 8 NeuronCores.

Computes mean(where(s == other_s, 1, -1) * x) for N = 2^25 elements.

Data-parallel across 8 cores; each core's shard is host-packed into a
compressed stream of 2.25 B/element (vs 12 B/element naive):
  - s, other_s are {0,1} -> bit-packed, 16 elements per int16 word
  - x -> fp16 (error on the final mean ~5e-4 relative, vs 2e-2 budget)

Layout per partition row (PFD = 32768 x elements):
  [ s_words 4096 B | o_words 4096 B | x planes: 16 x (2048 fp16) ]
where bit k of word j corresponds to x element 16j+k, stored in plane k
at offset j.

Device compute per plane k (the key trick: w = +-1 applied as an fp16
SIGN-BIT flip, so the mask never has to become an arithmetic value):
  u     = s ^ o                                  # int32 TT, once
  sgn_k = (u & ((1<<k)*0x00010001)) << (15-k)    # ts and+shl, int32 2x
  prod  = sgn_k XOR x_k                          # int16 TT xor, 2x_1p
          == where(s==o, x_k, -x_k)  exactly
  ACT sums prod pairs via activation(Copy, accum_out)  # off DVE
Host sums the 8 accumulator columns in f64 and divides by N.
"""

import contextlib
import ctypes
import os
import sys
import types

import numpy as np


def _install_ntff_hook_shim():
    """Register the axon NTFF-profile hook if the image's ``antenv`` lacks
    ``axon_hooks`` (boot degrades silently in that case, which breaks
    ``run_bass_kernel_spmd(trace=True)``)."""
    try:
        import antenv.axon_hooks  # noqa: F401

        return
    except ImportError:
        pass
    try:
        mod = types.ModuleType("antenv.axon_hooks")
        holder = {"hook": None}
        mod.set_axon_ntff_profile_hook = lambda h: holder.__setitem__("hook", h)
        mod.get_axon_ntff_profile_hook = lambda: holder["hook"]
        sys.modules["antenv.axon_hooks"] = mod
        try:
            import antenv

            antenv.axon_hooks = mod
        except ImportError:
            pass

        so_path = "/opt/axon/libaxon_pjrt.so"
        if not os.path.exists(so_path):
            return
        lib = ctypes.CDLL(so_path)
        if not hasattr(lib, "axon_start_nrt_profile"):
            return
        lib.axon_start_nrt_profile.argtypes = [
            ctypes.POINTER(ctypes.c_int64),
            ctypes.c_size_t,
        ]
        lib.axon_start_nrt_profile.restype = ctypes.c_int64
        lib.axon_stop_nrt_profile.argtypes = [ctypes.c_char_p]
        lib.axon_stop_nrt_profile.restype = ctypes.c_int64

        @contextlib.contextmanager
        def _hook(output_dir, device_ids):
            import jax

            jax.devices()
            if device_ids:
                ids = (ctypes.c_int64 * len(device_ids))(*device_ids)
                rc = lib.axon_start_nrt_profile(ids, len(device_ids))
            else:
                rc = lib.axon_start_nrt_profile(None, 0)
            if rc != 0:
                raise RuntimeError(f"axon_start_nrt_profile rc={rc}")
            try:
                yield
            finally:
                n = lib.axon_stop_nrt_profile(str(output_dir).encode())
                print(f"ntff profile: {n} file(s) -> {output_dir}", file=sys.stderr)

        holder["hook"] = _hook
    except Exception:
        pass


_install_ntff_hook_shim()

from concourse import bacc, mybir, tile
from concourse.bass_utils import run_bass_kernel_spmd

A = mybir.AluOpType

N = 33554432
NCORES = 8
PER = N // NCORES          # 4194304 elements per core
P = 128                    # SBUF partitions
PFD = PER // P             # 32768 x elements per partition
FB = PFD // 16             # 2048 elements per plane per partition
SOB = 2 * (PFD // 8)       # 8192 B/partition of s+o words
TOTAL_B = SOB + 2 * PFD    # 73728 B/partition

# Sub-DMA byte ranges per partition row: bit words first (unlock xor), then
# x planes — a small chunk first so compute starts early, small chunks last
# so little work remains after the final arrival.
_PLANE_CHUNKS = [2, 4, 4, 2, 2, 2]
assert sum(_PLANE_CHUNKS) == 16
_SUBS = [(0, SOB)]
_pbase = SOB
for _npl in _PLANE_CHUNKS:
    _SUBS.append((_pbase, _pbase + 2 * FB * _npl))
    _pbase += 2 * FB * _npl

_cache = {}


def _build():
    if "nc" in _cache:
        return _cache["nc"]

    nc = bacc.Bacc(
        "TRN2", target_bir_lowering=False, debug=False, num_devices=NCORES
    )

    sox = nc.dram_tensor(
        "sox", [P * TOTAL_B], mybir.dt.int8, kind="ExternalInput"
    )
    out = nc.dram_tensor("out", [P, 9], mybir.dt.float32, kind="ExternalOutput")

    with tile.TileContext(nc) as tc:
        with (
            tc.tile_pool(name="io", bufs=1) as io_pool,
            tc.tile_pool(name="sgn", bufs=6) as sgn_pool,
            tc.tile_pool(name="prod", bufs=4) as prod_pool,
            tc.tile_pool(name="stat", bufs=1) as stat_pool,
        ):
            acc = stat_pool.tile([P, 9], mybir.dt.float32)

            tl = io_pool.tile([P, TOTAL_B], mybir.dt.int8, tag="io")
            row = sox.ap().rearrange("(p f) -> p f", p=P)
            for lo, hi in _SUBS:
                nc.sync.dma_start(out=tl[:, lo:hi], in_=row[:, lo:hi])

            xr = sgn_pool.tile([P, SOB // 2], mybir.dt.int8, tag="xr")
            nc.vector.tensor_tensor(
                out=xr[:].bitcast(mybir.dt.int32),
                in0=tl[:, 0 : SOB // 2].bitcast(mybir.dt.int32),
                in1=tl[:, SOB // 2 : SOB].bitcast(mybir.dt.int32),
                op=A.bitwise_xor,
            )

            def xplane16(k):
                lo = SOB + 2 * k * FB
                return tl[:, lo : lo + 2 * FB].bitcast(mybir.dt.int16)

            for j in range(8):  # plane pairs
                pp = prod_pool.tile([P, 4 * FB], mybir.dt.int8, tag="pp")
                for h in range(2):
                    k = 2 * j + h
                    m = (1 << k) * 0x00010001
                    if m >= 1 << 31:
                        m -= 1 << 32
                    sg = sgn_pool.tile([P, 2 * FB], mybir.dt.int8, tag="sg")
                    nc.vector.tensor_scalar(
                        out=sg[:].bitcast(mybir.dt.int32),
                        in0=xr[:].bitcast(mybir.dt.int32),
                        scalar1=m,
                        scalar2=15 - k,
                        op0=A.bitwise_and,
                        op1=A.logical_shift_left,
                    )
                    nc.vector.tensor_tensor(
                        out=pp[:, 2 * h * FB : 2 * (h + 1) * FB].bitcast(
                            mybir.dt.int16
                        ),
                        in0=sg[:].bitcast(mybir.dt.int16),
                        in1=xplane16(k),
                        op=A.bitwise_xor,
                    )
                if j < 7:
                    nc.scalar.activation(
                        out=pp[:].bitcast(mybir.dt.float16),
                        in_=pp[:].bitcast(mybir.dt.float16),
                        func=mybir.ActivationFunctionType.Copy,
                        accum_out=acc[:, j : j + 1],
                    )
                else:
                    # Split the last pair's reduces: plane 14 on ACT, plane 15
                    # on DVE, so neither engine's serial chain sets the tail.
                    nc.scalar.activation(
                        out=pp[:, : 2 * FB].bitcast(mybir.dt.float16),
                        in_=pp[:, : 2 * FB].bitcast(mybir.dt.float16),
                        func=mybir.ActivationFunctionType.Copy,
                        accum_out=acc[:, 7:8],
                    )
                    tsout = stat_pool.tile(
                        [P, FB], mybir.dt.float16, tag="tsout"
                    )
                    nc.vector.tensor_scalar(
                        out=tsout[:],
                        in0=pp[:, 2 * FB :].bitcast(mybir.dt.float16),
                        scalar1=1.0,
                        scalar2=None,
                        op0=A.mult,
                        op1=A.add,
                        accum_out=acc[:, 8:9],
                    )

            nc.sync.dma_start(out=out[:], in_=acc[:])

    nc.compile()
    _cache["nc"] = nc
    return nc


def _pack(s, other_s, x):
    """Full inputs -> per-core compressed streams (list of int8 arrays)."""
    sb = np.packbits(
        s.astype(np.uint8).reshape(-1, 8), axis=1, bitorder="little"
    ).ravel()
    ob = np.packbits(
        other_s.astype(np.uint8).reshape(-1, 8), axis=1, bitorder="little"
    ).ravel()
    xh = x.astype(np.float16)

    bufs = []
    for c in range(NCORES):
        sq = sb[c * PER // 8 : (c + 1) * PER // 8].reshape(P, PFD // 8)
        oq = ob[c * PER // 8 : (c + 1) * PER // 8].reshape(P, PFD // 8)
        xq = (
            xh[c * PER : (c + 1) * PER]
            .reshape(P, FB, 16)
            .transpose(0, 2, 1)  # [P, plane, j]
            .copy()
            .view(np.uint8)
            .reshape(P, 2 * PFD)
        )
        blk = np.concatenate([sq.view(np.uint8), oq.view(np.uint8), xq], axis=1)
        bufs.append(np.ascontiguousarray(blk.reshape(-1)).view(np.int8))
    return bufs


def run(s, other_s, x, **spmd_kwargs):
    """Run on HW; returns (full_output, BassKernelResults)."""
    s = np.ascontiguousarray(np.asarray(s, dtype=np.int32).reshape(N))
    other_s = np.ascontiguousarray(np.asarray(other_s, dtype=np.int32).reshape(N))
    x = np.ascontiguousarray(np.asarray(x, dtype=np.float32).reshape(N))

    nc = _build()
    in_maps = [{"sox": b} for b in _pack(s, other_s, x)]
    res = run_bass_kernel_spmd(
        nc, in_maps, core_ids=list(range(NCORES)), **spmd_kwargs
    )

    total = 0.0
    for r in res.results:
        total += float(r["out"].astype(np.float64).sum())
    full = np.array(total / N, dtype=np.float32)
    return full, res


def kernel(s, other_s, x):
    out, _ = run(s, other_s, x)
    return out


# revision 33
# speedup vs baseline: 1.0524x; 1.0524x over previous
"""Discriminator-loss kernel for Trainium2, SPMD across 8 NeuronCores.

Computes mean(where(s == other_s, 1, -1) * x) for N = 2^25 elements.

Data-parallel across 8 cores; each core's shard is host-packed into a
compressed stream of 2.25 B/element (vs 12 B/element naive):
  - s, other_s are {0,1} -> bit-packed, 16 elements per int16 word
  - x -> fp16 (error on the final mean ~5e-4 relative, vs 2e-2 budget)

Layout per partition row (PFD = 32768 x elements):
  [ s_words 4096 B | o_words 4096 B | x planes: 16 x (2048 fp16) ]
where bit k of word j corresponds to x element 16j+k, stored in plane k
at offset j.

Device compute per plane k (the key trick: w = +-1 applied as an fp16
SIGN-BIT flip, so the mask never has to become an arithmetic value):
  u     = s ^ o                                  # int32 TT, once
  sgn_k = (u & ((1<<k)*0x00010001)) << (15-k)    # ts and+shl, int32 2x
  prod  = sgn_k XOR x_k                          # int16 TT xor, 2x_1p
          == where(s==o, x_k, -x_k)  exactly
  ACT sums prod pairs via activation(Copy, accum_out)  # off DVE
Host sums the 8 accumulator columns in f64 and divides by N.
"""

import contextlib
import ctypes
import os
import sys
import types

import numpy as np


def _install_ntff_hook_shim():
    """Register the axon NTFF-profile hook if the image's ``antenv`` lacks
    ``axon_hooks`` (boot degrades silently in that case, which breaks
    ``run_bass_kernel_spmd(trace=True)``)."""
    try:
        import antenv.axon_hooks  # noqa: F401

        return
    except ImportError:
        pass
    try:
        mod = types.ModuleType("antenv.axon_hooks")
        holder = {"hook": None}
        mod.set_axon_ntff_profile_hook = lambda h: holder.__setitem__("hook", h)
        mod.get_axon_ntff_profile_hook = lambda: holder["hook"]
        sys.modules["antenv.axon_hooks"] = mod
        try:
            import antenv

            antenv.axon_hooks = mod
        except ImportError:
            pass

        so_path = "/opt/axon/libaxon_pjrt.so"
        if not os.path.exists(so_path):
            return
        lib = ctypes.CDLL(so_path)
        if not hasattr(lib, "axon_start_nrt_profile"):
            return
        lib.axon_start_nrt_profile.argtypes = [
            ctypes.POINTER(ctypes.c_int64),
            ctypes.c_size_t,
        ]
        lib.axon_start_nrt_profile.restype = ctypes.c_int64
        lib.axon_stop_nrt_profile.argtypes = [ctypes.c_char_p]
        lib.axon_stop_nrt_profile.restype = ctypes.c_int64

        @contextlib.contextmanager
        def _hook(output_dir, device_ids):
            import jax

            jax.devices()
            if device_ids:
                ids = (ctypes.c_int64 * len(device_ids))(*device_ids)
                rc = lib.axon_start_nrt_profile(ids, len(device_ids))
            else:
                rc = lib.axon_start_nrt_profile(None, 0)
            if rc != 0:
                raise RuntimeError(f"axon_start_nrt_profile rc={rc}")
            try:
                yield
            finally:
                n = lib.axon_stop_nrt_profile(str(output_dir).encode())
                print(f"ntff profile: {n} file(s) -> {output_dir}", file=sys.stderr)

        holder["hook"] = _hook
    except Exception:
        pass


_install_ntff_hook_shim()

from concourse import bacc, mybir, tile
from concourse.bass_utils import run_bass_kernel_spmd

A = mybir.AluOpType

N = 33554432
NCORES = 8
PER = N // NCORES          # 4194304 elements per core
P = 128                    # SBUF partitions
PFD = PER // P             # 32768 x elements per partition
FB = PFD // 16             # 2048 elements per plane per partition
SOB = 2 * (PFD // 8)       # 8192 B/partition of s+o words
TOTAL_B = SOB + 2 * PFD    # 73728 B/partition

# Sub-DMA byte ranges per partition row: bit words first (unlock xor), then
# x planes — a small chunk first so compute starts early, small chunks last
# so little work remains after the final arrival.
_PLANE_CHUNKS = [2, 4, 4, 2, 2, 2]
assert sum(_PLANE_CHUNKS) == 16
_SUBS = [(0, SOB)]
_pbase = SOB
for _npl in _PLANE_CHUNKS:
    _SUBS.append((_pbase, _pbase + 2 * FB * _npl))
    _pbase += 2 * FB * _npl

_cache = {}


def _build():
    if "nc" in _cache:
        return _cache["nc"]

    nc = bacc.Bacc(
        "TRN2", target_bir_lowering=False, debug=False, num_devices=NCORES
    )

    sox = nc.dram_tensor(
        "sox", [P * TOTAL_B], mybir.dt.int8, kind="ExternalInput"
    )
    out = nc.dram_tensor("out", [P, 8], mybir.dt.float32, kind="ExternalOutput")

    with tile.TileContext(nc) as tc:
        with (
            tc.tile_pool(name="io", bufs=1) as io_pool,
            tc.tile_pool(name="sgn", bufs=6) as sgn_pool,
            tc.tile_pool(name="prod", bufs=4) as prod_pool,
            tc.tile_pool(name="stat", bufs=1) as stat_pool,
        ):
            acc = stat_pool.tile([P, 8], mybir.dt.float32)

            tl = io_pool.tile([P, TOTAL_B], mybir.dt.int8, tag="io")
            row = sox.ap().rearrange("(p f) -> p f", p=P)
            for lo, hi in _SUBS:
                nc.sync.dma_start(out=tl[:, lo:hi], in_=row[:, lo:hi])

            xr = sgn_pool.tile([P, SOB // 2], mybir.dt.int8, tag="xr")
            nc.vector.tensor_tensor(
                out=xr[:].bitcast(mybir.dt.int32),
                in0=tl[:, 0 : SOB // 2].bitcast(mybir.dt.int32),
                in1=tl[:, SOB // 2 : SOB].bitcast(mybir.dt.int32),
                op=A.bitwise_xor,
            )

            def xplane16(k):
                lo = SOB + 2 * k * FB
                return tl[:, lo : lo + 2 * FB].bitcast(mybir.dt.int16)

            for j in range(8):  # plane pairs
                pp = prod_pool.tile([P, 4 * FB], mybir.dt.int8, tag="pp")
                for h in range(2):
                    k = 2 * j + h
                    m = (1 << k) * 0x00010001
                    if m >= 1 << 31:
                        m -= 1 << 32
                    sg = sgn_pool.tile([P, 2 * FB], mybir.dt.int8, tag="sg")
                    nc.vector.tensor_scalar(
                        out=sg[:].bitcast(mybir.dt.int32),
                        in0=xr[:].bitcast(mybir.dt.int32),
                        scalar1=m,
                        scalar2=15 - k,
                        op0=A.bitwise_and,
                        op1=A.logical_shift_left,
                    )
                    nc.vector.tensor_tensor(
                        out=pp[:, 2 * h * FB : 2 * (h + 1) * FB].bitcast(
                            mybir.dt.int16
                        ),
                        in0=sg[:].bitcast(mybir.dt.int16),
                        in1=xplane16(k),
                        op=A.bitwise_xor,
                    )
                if j < 7:
                    nc.scalar.activation(
                        out=pp[:].bitcast(mybir.dt.float16),
                        in_=pp[:].bitcast(mybir.dt.float16),
                        func=mybir.ActivationFunctionType.Copy,
                        accum_out=acc[:, j : j + 1],
                    )
                else:
                    # Last pair reduced on DVE (one whole-pair accum) right
                    # after its TTs; ACT's serial chain ends pairs 0-6 at the
                    # same time, so neither engine sets the tail alone.
                    tsout = stat_pool.tile(
                        [P, 2 * FB], mybir.dt.float16, tag="tsout"
                    )
                    nc.vector.tensor_scalar(
                        out=tsout[:],
                        in0=pp[:].bitcast(mybir.dt.float16),
                        scalar1=1.0,
                        scalar2=None,
                        op0=A.mult,
                        op1=A.add,
                        accum_out=acc[:, 7:8],
                    )

            nc.sync.dma_start(out=out[:], in_=acc[:])

    nc.compile()
    _cache["nc"] = nc
    return nc


def _pack(s, other_s, x):
    """Full inputs -> per-core compressed streams (list of int8 arrays)."""
    sb = np.packbits(
        s.astype(np.uint8).reshape(-1, 8), axis=1, bitorder="little"
    ).ravel()
    ob = np.packbits(
        other_s.astype(np.uint8).reshape(-1, 8), axis=1, bitorder="little"
    ).ravel()
    xh = x.astype(np.float16)

    bufs = []
    for c in range(NCORES):
        sq = sb[c * PER // 8 : (c + 1) * PER // 8].reshape(P, PFD // 8)
        oq = ob[c * PER // 8 : (c + 1) * PER // 8].reshape(P, PFD // 8)
        xq = (
            xh[c * PER : (c + 1) * PER]
            .reshape(P, FB, 16)
            .transpose(0, 2, 1)  # [P, plane, j]
            .copy()
            .view(np.uint8)
            .reshape(P, 2 * PFD)
        )
        blk = np.concatenate([sq.view(np.uint8), oq.view(np.uint8), xq], axis=1)
        bufs.append(np.ascontiguousarray(blk.reshape(-1)).view(np.int8))
    return bufs


def run(s, other_s, x, **spmd_kwargs):
    """Run on HW; returns (full_output, BassKernelResults)."""
    s = np.ascontiguousarray(np.asarray(s, dtype=np.int32).reshape(N))
    other_s = np.ascontiguousarray(np.asarray(other_s, dtype=np.int32).reshape(N))
    x = np.ascontiguousarray(np.asarray(x, dtype=np.float32).reshape(N))

    nc = _build()
    in_maps = [{"sox": b} for b in _pack(s, other_s, x)]
    res = run_bass_kernel_spmd(
        nc, in_maps, core_ids=list(range(NCORES)), **spmd_kwargs
    )

    total = 0.0
    for r in res.results:
        total += float(r["out"].astype(np.float64).sum())
    full = np.array(total / N, dtype=np.float32)
    return full, res


def kernel(s, other_s, x):
    out, _ = run(s, other_s, x)
    return out
